# revision 13
# baseline (speedup 1.0000x reference)
"""Distributed Trainium2 kernel for the AND criterion (retrieval kNN loss).

Math: for L2-normalized rows zn of z [N, d], sim = zn @ zn.T,
logits = sim / T with the diagonal masked, and

  loss_i = -logsumexp_{j in top5}(log_softmax(logits)_ij)
         = log(sum_{j != i} exp(sim_ij/T)) - log(sum_{top5 j != i} exp(sim_ij/T))
  loss   = mean_i loss_i

Only top-5 *values* are needed (exp is monotonic) and the diagonal is always
the row max, so per row: top-8 of exp(sim/T) via the DVE max8 instruction
(rank 0 = self, ranks 1..5 = neighbors) + full-row exp-sum via the ScalarE
accumulator.

Implementation: rows sharded across 8 cores ([1024, 8192] sim block each,
full z^T replicated, host sums partial losses). The similarity matmuls run
in fp8e4m3 with DoubleRow packing (2 k-rows per PE cell): zn is scaled by
32/||z|| so values sit in fp8's normal range (~N(0,1)); the 1/1024 is folded
into the exp scale. Normalization runs in bf16 on a small rotating buffer
(squares -> ones-matmul over d -> ScalarE rsqrt -> scale), then casts into
the resident fp8 [128, 2, N] DoubleRow tiles on GpSimd. Column-pair-outer
sweep keeps the normalize feeder ahead of the TensorE.
"""

import numpy as np
import ml_dtypes
from contextlib import ExitStack

N = 8192
D = 1024
NCORES = 8
LOCAL = N // NCORES          # 1024 rows per core
P = 128
K_TILES = D // P             # 8 (bf16 view); 4 fp8 DoubleRow pair-tiles
T_TILES = K_TILES // 2       # 4
M_TILES = LOCAL // P         # 8
NC = 512
PAIR = 2 * NC                # 1024 columns processed per step
N_PAIRS = N // PAIR          # 8
EXP_SCALE = 10.0 / 1024.0    # 1/T, compensating the 32x fp8 scaling (32^2)

_CACHE = {}


def _build():
    import concourse.tile as tile
    import concourse.mybir as mybir
    from concourse import bacc

    dt = mybir.dt
    nc = bacc.Bacc(
        "TRN2", target_bir_lowering=False, debug=False, num_devices=NCORES
    )
    zt_d = nc.dram_tensor("zt", [D, N], dt.bfloat16, kind="ExternalInput")
    zl_d = nc.dram_tensor("zl", [D, LOCAL], dt.bfloat16, kind="ExternalInput")
    out_d = nc.dram_tensor("out", [P, M_TILES], dt.float32, kind="ExternalOutput")

    with tile.TileContext(nc) as tc:
        _body(tc, nc, mybir, zt_d, zl_d, out_d)

    nc.compile()
    return nc


def _body(tc, nc, mybir, zt_d, zl_d, out_d):
    dt = mybir.dt
    AF = mybir.ActivationFunctionType
    AX = mybir.AxisListType
    DR = mybir.MatmulPerfMode.DoubleRow

    with ExitStack() as ctx:
        ep = ctx.enter_context
        z8_pool = ep(tc.tile_pool(name="z8", bufs=T_TILES))
        zl8_pool = ep(tc.tile_pool(name="zl8", bufs=T_TILES))
        buf_pool = ep(tc.tile_pool(name="buf", bufs=4 * K_TILES))
        const_pool = ep(tc.tile_pool(name="const", bufs=1))
        sq_pool = ep(tc.tile_pool(name="sq", bufs=6))
        rn_pool = ep(tc.tile_pool(name="rn", bufs=4))
        exp_pool = ep(tc.tile_pool(name="exp", bufs=6))
        sums_pool = ep(tc.tile_pool(name="sums", bufs=M_TILES))
        cand_pool = ep(tc.tile_pool(name="cand", bufs=M_TILES))
        small_pool = ep(tc.tile_pool(name="small", bufs=4))
        res_pool = ep(tc.tile_pool(name="res", bufs=1))
        psum_pool = ep(tc.tile_pool(name="psum", bufs=4, space="PSUM"))

        ones = const_pool.tile([P, P], dt.bfloat16)
        nc.vector.memset(ones[:], 1.0)

        # fp8 DoubleRow operand stacks: [p, s, n] = zn32[d = 2t*128 + s*128 + p, n]
        zt8 = [z8_pool.tile([P, 2, N], dt.float8e4, name="zt8", tag="zt8")
               for _ in range(T_TILES)]
        zl8 = [zl8_pool.tile([P, 2, LOCAL], dt.float8e4, name="zl8", tag="zl8")
               for _ in range(T_TILES)]

        # Normalize one column-pair arriving in bufs[k] [128, PAIR] bf16, then
        # cast into the fp8 stacks. nrm2 broadcast to all partitions via
        # ones-matmul over d; 32/nrm via one ScalarE rsqrt of nrm2/1024.
        def feed_pair(bufs, dst8, c):
            ps = psum_pool.tile([P, PAIR], dt.float32, name="ps", tag="ps")
            for k in range(K_TILES):
                sq = sq_pool.tile([P, PAIR], dt.bfloat16, name="sq", tag="sq")
                nc.vector.tensor_mul(sq[:], bufs[k][:], bufs[k][:])
                for h in range(2):
                    nc.tensor.matmul(
                        ps[:, h * NC:(h + 1) * NC], lhsT=ones[:],
                        rhs=sq[:, h * NC:(h + 1) * NC],
                        start=(k == 0), stop=(k == K_TILES - 1),
                    )
            rnb = rn_pool.tile([P, PAIR], dt.bfloat16, name="rnb", tag="rnb")
            nc.scalar.activation(rnb[:], ps[:], AF.Abs_reciprocal_sqrt,
                                 scale=1.0 / 1024.0)
            cs = slice(c * PAIR, (c + 1) * PAIR)
            for k in range(K_TILES):
                nc.vector.tensor_mul(dst8[k // 2][:, k % 2, cs],
                                     bufs[k][:], rnb[:])

        # ---- local z^T -> zl8 ----
        zlb = []
        for k in range(K_TILES):
            t = buf_pool.tile([P, PAIR], dt.bfloat16, name="buf", tag="buf")
            nc.sync.dma_start(out=t[:], in_=zl_d[k * P:(k + 1) * P, :])
            zlb.append(t)
        feed_pair(zlb, zl8, 0)

        # ---- stats tiles (live across the whole sweep) ----
        sums = [sums_pool.tile([P, N_PAIRS], dt.float32, name="sums", tag="sums")
                for _ in range(M_TILES)]
        cand = [cand_pool.tile([P, N_PAIRS * 8], dt.bfloat16, name="cand",
                               tag="cand") for _ in range(M_TILES)]

        # ---- similarity sweep: pair outer, row-tile inner ----
        # The feeder for a pair is emitted ahead of the previous pairs'
        # sweeps (2-pair software pipeline) so its DVE/ACT work outranks the
        # epilogues in the scheduler, and consecutive rsqrts share one ACT
        # table load.
        def load_pair(pr):
            cs = slice(pr * PAIR, (pr + 1) * PAIR)
            bufs = []
            for k in range(K_TILES):
                t = buf_pool.tile([P, PAIR], dt.bfloat16, name="buf", tag="buf")
                nc.sync.dma_start(out=t[:], in_=zt_d[k * P:(k + 1) * P, cs])
                bufs.append(t)
            feed_pair(bufs, zt8, pr)

        load_pair(0)
        load_pair(1)
        for pr in range(N_PAIRS):
            if pr % 2 == 0 and pr + 2 < N_PAIRS:
                load_pair(pr + 2)
                load_pair(pr + 3)

            for mt in range(M_TILES):
                ms = slice(mt * P, (mt + 1) * P)
                ps = psum_pool.tile([P, PAIR], dt.float32, name="ps", tag="ps")
                for t in range(T_TILES):
                    for h in range(2):
                        hs = slice(pr * PAIR + h * NC, pr * PAIR + (h + 1) * NC)
                        nc.tensor.matmul(
                            ps[:, h * NC:(h + 1) * NC],
                            lhsT=zl8[t][:, :, ms], rhs=zt8[t][:, :, hs],
                            start=(t == 0), stop=(t == T_TILES - 1),
                            perf_mode=DR,
                        )
                ex = exp_pool.tile([P, PAIR], dt.bfloat16, name="ex", tag="ex")
                nc.scalar.activation(
                    ex[:], ps[:], AF.Exp,
                    scale=EXP_SCALE, accum_out=sums[mt][:, pr:pr + 1],
                )
                nc.vector.max(out=cand[mt][:, pr * 8:(pr + 1) * 8], in_=ex[:])

        # ---- per-row finalize ----
        sall_all = res_pool.tile([P, M_TILES], dt.float32)
        s5_all = res_pool.tile([P, M_TILES], dt.float32)
        for mt in range(M_TILES):
            top8 = small_pool.tile([P, 8], dt.bfloat16, name="top8", tag="top8")
            nc.vector.max(out=top8[:], in_=cand[mt][:])
            nc.vector.reduce_sum(s5_all[:, mt:mt + 1], top8[:, 1:6], AX.X)
            big = small_pool.tile([P, 1], dt.float32, name="big", tag="big")
            nc.vector.reduce_sum(big[:], sums[mt][:], AX.X)
            nc.vector.tensor_sub(sall_all[:, mt:mt + 1], big[:], top8[:, 0:1])

        lna = res_pool.tile([P, M_TILES], dt.float32)
        ln5 = res_pool.tile([P, M_TILES], dt.float32)
        nc.scalar.activation(lna[:], sall_all[:], AF.Ln)
        nc.scalar.activation(ln5[:], s5_all[:], AF.Ln)
        losses = res_pool.tile([P, M_TILES], dt.float32)
        nc.vector.tensor_sub(losses[:], lna[:], ln5[:])
        nc.sync.dma_start(out=out_d[:, :], in_=losses[:])


def _get_nc():
    if "nc" not in _CACHE:
        _CACHE["nc"] = _build()
    return _CACHE["nc"]


def _run(z, trace=False):
    from concourse.bass_utils import run_bass_kernel_spmd

    zt = np.ascontiguousarray(z.T).astype(ml_dtypes.bfloat16)  # [D, N]
    in_maps = [
        {"zt": zt, "zl": np.ascontiguousarray(zt[:, i * LOCAL:(i + 1) * LOCAL])}
        for i in range(NCORES)
    ]
    nc = _get_nc()
    res = run_bass_kernel_spmd(
        nc, in_maps, core_ids=list(range(NCORES)), trace=trace
    )
    total = np.float64(0.0)
    for i in range(NCORES):
        total += np.asarray(res.results[i]["out"], dtype=np.float64).sum()
    loss = np.array(total / N, dtype=np.float32)
    return loss, res


def kernel(z):
    loss, _ = _run(np.asarray(z, dtype=np.float32), trace=False)
    return loss


def bench(z, trace=True):
    loss, res = _run(np.asarray(z, dtype=np.float32), trace=trace)
    return loss, res


# revision 18
# speedup vs baseline: 1.0232x; 1.0232x over previous
"""Distributed Trainium2 kernel for the AND criterion (retrieval kNN loss).

Math: for L2-normalized rows zn of z [N, d], sim = zn @ zn.T,
logits = sim / T with the diagonal masked, and

  loss_i = -logsumexp_{j in top5}(log_softmax(logits)_ij)
         = log(sum_{j != i} exp(sim_ij/T)) - log(sum_{top5 j != i} exp(sim_ij/T))
  loss   = mean_i loss_i

Only top-5 *values* are needed (exp is monotonic) and the diagonal is always
the row max, so per row: top-8 of exp(sim/T) via the DVE max8 instruction
(rank 0 = self, ranks 1..5 = neighbors) + full-row exp-sum via the ScalarE
accumulator.

Implementation: rows sharded across 8 cores ([1024, 8192] sim block each,
full z^T replicated, host sums partial losses). The similarity matmuls run
in fp8e4m3 with DoubleRow packing (2 k-rows per PE cell): zn is scaled by
32/||z|| so values sit in fp8's normal range (~N(0,1)); the 1/1024 is folded
into the exp scale. Normalization runs in bf16 on a small rotating buffer
(squares -> ones-matmul over d -> ScalarE rsqrt -> scale), then casts into
the resident fp8 [128, 2, N] DoubleRow tiles on GpSimd. Column-pair-outer
sweep keeps the normalize feeder ahead of the TensorE.
"""

import numpy as np
import ml_dtypes
from contextlib import ExitStack

N = 8192
D = 1024
NCORES = 8
LOCAL = N // NCORES          # 1024 rows per core
P = 128
K_TILES = D // P             # 8 (bf16 view); 4 fp8 DoubleRow pair-tiles
T_TILES = K_TILES // 2       # 4
M_TILES = LOCAL // P         # 8
NC = 512
PAIR = 2 * NC                # 1024 columns processed per step
N_PAIRS = N // PAIR          # 8
EXP_SCALE = 10.0 / 1024.0    # 1/T, compensating the 32x fp8 scaling (32^2)

_CACHE = {}


def _build():
    import concourse.tile as tile
    import concourse.mybir as mybir
    from concourse import bacc

    dt = mybir.dt
    nc = bacc.Bacc(
        "TRN2", target_bir_lowering=False, debug=False, num_devices=NCORES
    )
    zt_d = nc.dram_tensor("zt", [D, N], dt.bfloat16, kind="ExternalInput")
    zl_d = nc.dram_tensor("zl", [D, LOCAL], dt.bfloat16, kind="ExternalInput")
    out_d = nc.dram_tensor("out", [P, M_TILES], dt.float32, kind="ExternalOutput")

    with tile.TileContext(nc) as tc:
        _body(tc, nc, mybir, zt_d, zl_d, out_d)

    nc.compile()
    return nc


def _body(tc, nc, mybir, zt_d, zl_d, out_d):
    dt = mybir.dt
    AF = mybir.ActivationFunctionType
    AX = mybir.AxisListType
    DR = mybir.MatmulPerfMode.DoubleRow

    with ExitStack() as ctx:
        ep = ctx.enter_context
        z8_pool = ep(tc.tile_pool(name="z8", bufs=T_TILES))
        zl8_pool = ep(tc.tile_pool(name="zl8", bufs=T_TILES))
        buf_pool = ep(tc.tile_pool(name="buf", bufs=4 * K_TILES))
        const_pool = ep(tc.tile_pool(name="const", bufs=1))
        sq_pool = ep(tc.tile_pool(name="sq", bufs=10))
        rn_pool = ep(tc.tile_pool(name="rn", bufs=4))
        exp_pool = ep(tc.tile_pool(name="exp", bufs=6))
        sums_pool = ep(tc.tile_pool(name="sums", bufs=M_TILES))
        cand_pool = ep(tc.tile_pool(name="cand", bufs=M_TILES))
        small_pool = ep(tc.tile_pool(name="small", bufs=4))
        res_pool = ep(tc.tile_pool(name="res", bufs=1))
        psum_pool = ep(tc.tile_pool(name="psum", bufs=4, space="PSUM"))

        ones = const_pool.tile([P, P], dt.bfloat16)
        nc.vector.memset(ones[:], 1.0)

        # fp8 DoubleRow operand stacks: [p, s, n] = zn32[d = 2t*128 + s*128 + p, n]
        zt8 = [z8_pool.tile([P, 2, N], dt.float8e4, name="zt8", tag="zt8")
               for _ in range(T_TILES)]
        zl8 = [zl8_pool.tile([P, 2, LOCAL], dt.float8e4, name="zl8", tag="zl8")
               for _ in range(T_TILES)]

        # Normalize one column-pair arriving in bufs[k] [128, PAIR] bf16, then
        # cast into the fp8 stacks. nrm2 broadcast to all partitions via
        # ones-matmul over d; 32/nrm via one ScalarE rsqrt of nrm2/1024.
        def feed_pair(bufs, dst8, c):
            ps = psum_pool.tile([P, PAIR], dt.float32, name="ps", tag="ps")
            for k in range(K_TILES):
                sq = sq_pool.tile([P, PAIR], dt.bfloat16, name="sq", tag="sq")
                nc.vector.tensor_mul(sq[:], bufs[k][:], bufs[k][:])
                for h in range(2):
                    nc.tensor.matmul(
                        ps[:, h * NC:(h + 1) * NC], lhsT=ones[:],
                        rhs=sq[:, h * NC:(h + 1) * NC],
                        start=(k == 0), stop=(k == K_TILES - 1),
                    )
            rnb = rn_pool.tile([P, PAIR], dt.bfloat16, name="rnb", tag="rnb")
            nc.scalar.activation(rnb[:], ps[:], AF.Abs_reciprocal_sqrt,
                                 scale=1.0 / 1024.0)
            cs = slice(c * PAIR, (c + 1) * PAIR)
            for k in range(K_TILES):
                sc = sq_pool.tile([P, PAIR], dt.bfloat16, name="sq", tag="sq")
                nc.vector.tensor_mul(sc[:], bufs[k][:], rnb[:])
                nc.vector.tensor_copy(dst8[k // 2][:, k % 2, cs], sc[:])

        # ---- local z^T -> zl8 ----
        zlb = []
        for k in range(K_TILES):
            t = buf_pool.tile([P, PAIR], dt.bfloat16, name="buf", tag="buf")
            nc.sync.dma_start(out=t[:], in_=zl_d[k * P:(k + 1) * P, :])
            zlb.append(t)
        feed_pair(zlb, zl8, 0)

        # ---- stats tiles (live across the whole sweep) ----
        sums = [sums_pool.tile([P, N_PAIRS], dt.float32, name="sums", tag="sums")
                for _ in range(M_TILES)]
        cand = [cand_pool.tile([P, N_PAIRS * 4], dt.bfloat16, name="cand",
                               tag="cand") for _ in range(M_TILES)]
        # exp outputs for two consecutive pairs share one tile so max8 runs
        # on [P, 2*PAIR] (fewer DVE per-op bubbles); top-8 of a 2048-wide
        # chunk still contains its top-5.
        ex2 = [None] * M_TILES

        # ---- similarity sweep: pair outer, row-tile inner ----
        # The feeder for a pair is emitted ahead of the previous pairs'
        # sweeps (2-pair software pipeline) so its DVE/ACT work outranks the
        # epilogues in the scheduler, and consecutive rsqrts share one ACT
        # table load.
        def load_pair(pr):
            cs = slice(pr * PAIR, (pr + 1) * PAIR)
            bufs = []
            for k in range(K_TILES):
                t = buf_pool.tile([P, PAIR], dt.bfloat16, name="buf", tag="buf")
                nc.sync.dma_start(out=t[:], in_=zt_d[k * P:(k + 1) * P, cs])
                bufs.append(t)
            feed_pair(bufs, zt8, pr)

        load_pair(0)
        load_pair(1)
        for pr in range(N_PAIRS):
            if pr % 2 == 0 and pr + 2 < N_PAIRS:
                load_pair(pr + 2)
                load_pair(pr + 3)

            for mt in range(M_TILES):
                ms = slice(mt * P, (mt + 1) * P)
                ps = psum_pool.tile([P, PAIR], dt.float32, name="ps", tag="ps")
                for t in range(T_TILES):
                    for h in range(2):
                        hs = slice(pr * PAIR + h * NC, pr * PAIR + (h + 1) * NC)
                        nc.tensor.matmul(
                            ps[:, h * NC:(h + 1) * NC],
                            lhsT=zl8[t][:, :, ms], rhs=zt8[t][:, :, hs],
                            start=(t == 0), stop=(t == T_TILES - 1),
                            perf_mode=DR,
                        )
                if pr % 2 == 0:
                    ex2[mt] = exp_pool.tile([P, 2 * PAIR], dt.bfloat16,
                                            name="ex", tag="ex")
                nc.scalar.activation(
                    ex2[mt][:, (pr % 2) * PAIR:(pr % 2 + 1) * PAIR],
                    ps[:], AF.Exp,
                    scale=EXP_SCALE, accum_out=sums[mt][:, pr:pr + 1],
                )
                if pr % 2 == 1:
                    pp = pr // 2
                    nc.vector.max(out=cand[mt][:, pp * 8:(pp + 1) * 8],
                                  in_=ex2[mt][:])

        # ---- per-row finalize ----
        sall_all = res_pool.tile([P, M_TILES], dt.float32)
        s5_all = res_pool.tile([P, M_TILES], dt.float32)
        for mt in range(M_TILES):
            top8 = small_pool.tile([P, 8], dt.bfloat16, name="top8", tag="top8")
            nc.vector.max(out=top8[:], in_=cand[mt][:])
            nc.vector.reduce_sum(s5_all[:, mt:mt + 1], top8[:, 1:6], AX.X)
            big = small_pool.tile([P, 1], dt.float32, name="big", tag="big")
            nc.vector.reduce_sum(big[:], sums[mt][:], AX.X)
            nc.vector.tensor_sub(sall_all[:, mt:mt + 1], big[:], top8[:, 0:1])

        lna = res_pool.tile([P, M_TILES], dt.float32)
        ln5 = res_pool.tile([P, M_TILES], dt.float32)
        nc.scalar.activation(lna[:], sall_all[:], AF.Ln)
        nc.scalar.activation(ln5[:], s5_all[:], AF.Ln)
        losses = res_pool.tile([P, M_TILES], dt.float32)
        nc.vector.tensor_sub(losses[:], lna[:], ln5[:])
        nc.sync.dma_start(out=out_d[:, :], in_=losses[:])


def _get_nc():
    if "nc" not in _CACHE:
        _CACHE["nc"] = _build()
    return _CACHE["nc"]


def _run(z, trace=False):
    from concourse.bass_utils import run_bass_kernel_spmd

    zt = np.ascontiguousarray(z.T).astype(ml_dtypes.bfloat16)  # [D, N]
    in_maps = [
        {"zt": zt, "zl": np.ascontiguousarray(zt[:, i * LOCAL:(i + 1) * LOCAL])}
        for i in range(NCORES)
    ]
    nc = _get_nc()
    res = run_bass_kernel_spmd(
        nc, in_maps, core_ids=list(range(NCORES)), trace=trace
    )
    total = np.float64(0.0)
    for i in range(NCORES):
        total += np.asarray(res.results[i]["out"], dtype=np.float64).sum()
    loss = np.array(total / N, dtype=np.float32)
    return loss, res


def kernel(z):
    loss, _ = _run(np.asarray(z, dtype=np.float32), trace=False)
    return loss


def bench(z, trace=True):
    loss, res = _run(np.asarray(z, dtype=np.float32), trace=trace)
    return loss, res
